# revision 30
# baseline (speedup 1.0000x reference)
"""Trainium2 Bass kernel for a ResNet BasicBlock (dense CNN, sync-BN).

Reference computation (training-mode BN, batch stats over (N,H,W)):
    h = conv3x3(x, W1) * mask1            # structured channel pruning
    h = relu(bn(h, gamma1, beta1))
    h = conv3x3(h, W2) * mask2
    h = bn(h, gamma2, beta2)
    out = relu(h + x)                      # identity shortcut

Shapes: x [32, 256, 56, 56] f32, W [256, 256, 3, 3] f32.

Strategy: data-parallel over batch N across 8 NeuronCores (4 images per
core), weights replicated, BN stats synchronized with tiny remote-DMA
all-broadcasts ([*,2] per-channel sum/sumsq).

Mask specialization (compiled per mask pattern): only the K1 = |mask1|
live channels of conv1 are computed.  Since K1 > 128:
  - conv1 runs in *transposed* mode: stationary = a 128-position slice
    of the padded x plane, moving = W1 gathered to [ci, K1]; one PSUM
    chunk is [128 positions, K1 channels] and costs K1 (not 448*2)
    PE-cycles per (ci-group, tap).  Output chunks cover 2 padded rows
    (116 positions) so the interior copy-out is row-aligned; stationary
    stays 128-wide (FWL) by overcomputing 12 positions.
  - h1 is transposed back to channel-major padded planes with PE
    transposes (two per chunk: channels 0..127 and the G2 overflow),
    BN1 stats accumulate during the interior copy-out (ACT).
  - conv2 runs in normal mode with *home-layout* output (masked output
    channels get zero weights), contraction over the 137 live h1
    channels as one 128-group (9 taps) plus one 81-row im2col group
    (G2 overflow channels x 9 taps, materialized by 9 SBUF->SBUF
    shift-DMAs per image): 10 matmuls per chunk instead of 18.
BN2 and the residual tail are identical to the dense kernel (masked
channels have h2 = 0 exactly, so bn2 -> beta2 - 0 and out = relu(x)).
"""

import numpy as np
import ml_dtypes

# ---- problem constants (hardcoded; kernel.py must be self-contained) ----
N_TOT, C, H, W = 32, 256, 56, 56
N_CORES = 8
NL = N_TOT // N_CORES          # images per core
PW = H + 2                     # padded row stride (58)
MARG = 64                      # front/back zero margin for shifted reads
PPLANE = PW * PW               # 3364 padded positions
PLANE_T = MARG + PPLANE + MARG # full plane tile length (3492)
STRIP0 = PW + 1                # first interior position (59), plane-relative
CHUNK2 = 8 * PW                # 464: conv2 processes 8 output rows per chunk
NCHUNK2 = 7
NCHUNK1 = 28                   # conv1-T: 28 chunks x 2 rows
C1P = 2 * PW                   # 116 positions per conv1-T chunk
IMLEN = 3368                   # im2col valid length (conv2 reads q in [59,3307))
HW = H * W                     # 3136
HALF_ROWS = 14                 # row granularity for x/out streaming DMAs
HALF_ELEMS = HALF_ROWS * W
NHALF = H // HALF_ROWS         # 4
COUNT = N_TOT * HW             # sync-BN element count per channel
EPS = 1e-5

_BF16 = ml_dtypes.bfloat16
_TAPS = [(ky, kx) for ky in range(3) for kx in range(3)]

_cache = {}


def _pack(W1, W2, mask1, mask2, gamma1, beta1, gamma2, beta2):
    S1 = np.nonzero(np.asarray(mask1) != 0)[0]
    K1 = len(S1)
    assert 128 < K1 <= 256, f"kernel specialized for 128<K1<=256, got {K1}"
    G2 = K1 - 128
    g1, g2 = S1[:128], S1[128:]

    W1 = np.asarray(W1, np.float32)
    W2m = (np.asarray(W2, np.float32)
           * np.asarray(mask2, np.float32)[:, None, None, None])

    # conv1 stationary tiles: og1 [ci, (cig,tap) x g1-chans],
    # og2 [ci, (cig,tap) x g2-chans]  (normal mode, gathered outputs)
    wt1a = np.empty((128, 18 * 128), np.float32)
    for cig in range(2):
        for t, (ky, kx) in enumerate(_TAPS):
            blk = cig * 9 + t
            wt1a[:, blk * 128:(blk + 1) * 128] = \
                W1[g1, cig * 128:(cig + 1) * 128, ky, kx].T
    # og2 dual: tap-partials P[(t,c),q] = sum_ci x[ci,q] W1[g2[c],ci,t];
    # combined later with shifted selector matmuls
    wt1b = np.empty((128, 2 * 9 * G2), np.float32)
    for cig in range(2):
        for t, (ky, kx) in enumerate(_TAPS):
            wt1b[:, cig * 9 * G2 + t * G2:cig * 9 * G2 + (t + 1) * G2] = \
                W1[g2, cig * 128:(cig + 1) * 128, ky, kx].T
    sel = np.eye(9 * G2, dtype=np.float32)

    # conv2 stationary, g1 group: [ci=h1 partition i (ch g1[i]), co home]
    wt2a = np.empty((128, 18 * 128), np.float32)
    for j in range(2):
        for t, (ky, kx) in enumerate(_TAPS):
            blk = j * 9 + t
            wt2a[:, blk * 128:(blk + 1) * 128] = \
                W2m[j * 128:(j + 1) * 128, :, ky, kx][:, g1].T

    # conv2 stationary, im2col group: row di*G2+c -> (tap di, ch g2[c])
    wt2b = np.empty((9 * G2, 2 * 128), np.float32)
    for j in range(2):
        for di, (ky, kx) in enumerate(_TAPS):
            wt2b[di * G2:(di + 1) * G2, j * 128:(j + 1) * 128] = \
                W2m[j * 128:(j + 1) * 128, :, ky, kx][:, g2].T

    # BN affine params: BN1 in gathered order, BN2 in home order
    aff = np.zeros((128, 8), np.float32)
    aff[:, 0] = np.asarray(gamma1, np.float32)[g1]
    aff[:G2, 1] = np.asarray(gamma1, np.float32)[g2]
    aff[:, 2] = np.asarray(beta1, np.float32)[g1]
    aff[:G2, 3] = np.asarray(beta1, np.float32)[g2]
    aff[:, 4] = np.asarray(gamma2, np.float32).reshape(2, 128)[0]
    aff[:, 5] = np.asarray(gamma2, np.float32).reshape(2, 128)[1]
    aff[:, 6] = np.asarray(beta2, np.float32).reshape(2, 128)[0]
    aff[:, 7] = np.asarray(beta2, np.float32).reshape(2, 128)[1]

    return (K1, wt1a.astype(_BF16), wt1b.astype(_BF16), sel.astype(_BF16),
            wt2a.astype(_BF16), wt2b.astype(_BF16), aff)


def _build(K1):
    import concourse.bass as bass_mod
    import concourse.bacc as bacc
    import concourse.mybir as mybir
    import concourse.tile as tile

    G2 = K1 - 128
    IC2 = 9 * G2

    f32 = mybir.dt.float32
    bf16 = mybir.dt.bfloat16
    AX = mybir.AxisListType
    ALU = mybir.AluOpType
    AF = mybir.ActivationFunctionType

    nc = bacc.Bacc("TRN2", target_bir_lowering=False, debug=False,
                   num_devices=N_CORES)

    x_d = nc.dram_tensor("x", [NL, C, H, W], f32, kind="ExternalInput")
    wt1a_d = nc.dram_tensor("wt1a", [128, 18 * 128], bf16, kind="ExternalInput")
    wt1b_d = nc.dram_tensor("wt1b", [128, 2 * 9 * G2], bf16, kind="ExternalInput")
    sel_d = nc.dram_tensor("sel", [IC2, IC2], bf16, kind="ExternalInput")
    wt2a_d = nc.dram_tensor("wt2a", [128, 18 * 128], bf16, kind="ExternalInput")
    wt2b_d = nc.dram_tensor("wt2b", [IC2, 2 * 128], bf16, kind="ExternalInput")
    aff_d = nc.dram_tensor("aff", [128, 8], f32, kind="ExternalInput")
    out_d = nc.dram_tensor("out", [NL, C, H, W], f32, kind="ExternalOutput")

    groups = [list(range(N_CORES))]

    def rows(tile_ap, base, nrows, off=0):
        """[P, nrows, 58] view of whole padded rows starting at plane
        offset `base` (row-aligned, absolute incl. margin)."""
        v = tile_ap[:, base + off:base + off + nrows * PW].rearrange(
            "p (r c) -> p r c", c=PW)
        return v

    with tile.TileContext(nc) as tc:
        import contextlib
        with contextlib.ExitStack() as ctx:
            const = ctx.enter_context(tc.tile_pool(name="const", bufs=1))
            psA = ctx.enter_context(tc.tile_pool(name="psA", bufs=6, space="PSUM"))
            xst = ctx.enter_context(tc.tile_pool(name="xst", bufs=3))

            wt1a_sb = const.tile([128, 18 * 128], bf16, tag="wt1a", name="wt1a")
            nc.sync.dma_start(wt1a_sb[:], wt1a_d[:])
            wt1b_sb = const.tile([128, 2 * 9 * G2], bf16, tag="wt1b", name="wt1b")
            nc.sync.dma_start(wt1b_sb[:], wt1b_d[:])
            sel_sb = const.tile([IC2, IC2], bf16, tag="sel", name="sel")
            nc.sync.dma_start(sel_sb[:], sel_d[:])
            wt2a_sb = const.tile([128, 18 * 128], bf16, tag="wt2a", name="wt2a")
            nc.sync.dma_start(wt2a_sb[:], wt2a_d[:])
            wt2b_sb = const.tile([IC2, 2 * 128], bf16, tag="wt2b", name="wt2b")
            nc.sync.dma_start(wt2b_sb[:], wt2b_d[:])
            aff_sb = const.tile([128, 8], f32, tag="aff", name="aff")
            nc.sync.dma_start(aff_sb[:], aff_d[:])

            # ---- cross-core stats exchange plumbing (SBUF remote DMA) ----
            rsem = [nc.alloc_semaphore(f"rst{i}") for i in range(4)]
            lsem = nc.alloc_semaphore("lst")
            _gp_prev = [None]
            deferred_waits = []

            def gp_order(bi):
                if _gp_prev[0] is not None:
                    bass_mod._add_dep_helper(bi.ins, _gp_prev[0].ins,
                                             sync=False,
                                             reason="stats-exchange order")
                _gp_prev[0] = bi
                return bi

            nc._bir_kernel_barrier_sem_replica_groups.extend(
                set(g) for g in groups)

            def defer_wait(bi, sem, val):
                bi._wait_ge(sem, 0)
                deferred_waits.append((bi, sem, val))
                return bi

            for i, s in enumerate(rsem + [lsem]):
                cl = gp_order(nc.gpsimd.sem_clear(s))
                if i == 0:
                    defer_wait(cl, nc._bir_kernel_barrier_sem,
                               nc.bir_kernel_barrier_sem_inc)

            # persistent per-image planes
            x_pad = [[const.tile([128, PLANE_T], bf16, tag=f"xp{j}_{n}",
                                 name=f"xp{j}_{n}")
                      for n in range(NL)] for j in range(2)]
            h1g1 = [const.tile([128, PLANE_T], bf16, tag=f"h1_{n}",
                               name=f"h1_{n}") for n in range(NL)]
            g2pack = const.tile([G2, NL * PLANE_T], bf16, tag="g2p", name="g2p")
            im2col = const.tile([IC2, 2 * IMLEN], bf16, tag="i2c", name="i2c")
            h2 = [[const.tile([128, HW], bf16, tag=f"h2{j}_{n}",
                              name=f"h2{j}_{n}")
                   for n in range(NL)] for j in range(2)]

            # zero only the non-interior plane structure (front margin +
            # top pad row, the 56 pad-column pairs, bottom pad row + back
            # margin); x planes of image 0 first so conv1 starts early
            def zero_pads(t, base, P=128):
                nc.vector.memset(t[0:P, base:base + MARG + STRIP0], 0.0)
                pairs = t[0:P, base + MARG + 2 * PW - 1:
                          base + MARG + 2 * PW - 1 + 56 * PW].rearrange(
                    "p (r c) -> p r c", c=PW)[:, :, 0:2]
                nc.vector.memset(pairs, 0.0)
                nc.vector.memset(
                    t[0:P, base + MARG + STRIP0 + 56 * PW:base + PLANE_T], 0.0)
            def head_img(n):
                for j in range(2):
                    for rh in range(NHALF):
                        r0 = rh * HALF_ROWS
                        xs = xst.tile([128, HALF_ELEMS], f32, tag="xs",
                                      name="xs")
                        nc.sync.dma_start(
                            xs[:],
                            x_d[n, j * 128:(j + 1) * 128,
                                r0:r0 + HALF_ROWS, :])
                        dst = rows(x_pad[j][n], MARG + (r0 + 1) * PW,
                                   HALF_ROWS)[:, :, 1:57]
                        src = xs[:, :].rearrange("p (r c) -> p r c", c=W)
                        nc.vector.tensor_copy(dst, src)
            for j in range(2):
                zero_pads(x_pad[j][0], 0)
            head_img(0)
            for n in range(1, NL):
                for j in range(2):
                    zero_pads(x_pad[j][n], 0)
            for n in range(NL):
                zero_pads(h1g1[n], 0)
                zero_pads(g2pack, n * PLANE_T, P=G2)
            zero_pads(pplane, 0, P=IC2)

            # stats accumulators: one column per (image, chunk)
            a1s = const.tile([128, NL * NCHUNK2], f32, tag="a1s", name="a1s")
            a1q = const.tile([128, NL * NCHUNK2], f32, tag="a1q", name="a1q")
            a1s2 = const.tile([G2, NL * NCHUNK2], f32, tag="a1s2", name="a1s2")
            a1q2 = const.tile([G2, NL * NCHUNK2], f32, tag="a1q2", name="a1q2")
            a2 = {(s, j): const.tile([128, NL * NCHUNK2], f32,
                                     tag=f"a2{s}{j}", name=f"a2{s}{j}")
                  for s in ("s", "q") for j in range(2)}

            # ---- head: stream the remaining images in ----
            for n in range(1, NL):
                head_img(n)

            # ---- conv1: og1 direct (18 MM/chunk); og2 via dual im2col
            # (2 partial MMs + 9 shifted selector MMs per chunk) ----
            c1_last = [None]
            c1a_last = [None]
            KB = (range(0, 4), range(4, 7))
            # og1 phase (all images) -- its stats are exchanged while the
            # og2 phase still computes, hiding one sync round-trip
            for n in range(NL):
                for kb in KB:
                    pss = {k: psA.tile([128, 448], f32, tag="ps", name="ps")
                           for k in kb}
                    for idx, (cig, (t, (ky, kx))) in enumerate(
                            (c, tt) for c in range(2)
                            for tt in enumerate(_TAPS)):
                        d = (ky - 1) * PW + (kx - 1)
                        blk = cig * 9 + t
                        for k in kb:
                            off = MARG + STRIP0 + CHUNK2 * k + d
                            rhs = x_pad[cig][n][:, off:off + CHUNK2
                                                ].rearrange(
                                "p (r c) -> p r c", c=PW)[:, :, 0:56]
                            nc.tensor.matmul(
                                pss[k][0:128, :],
                                wt1a_sb[:, blk * 128:(blk + 1) * 128],
                                rhs, start=(idx == 0), stop=(idx == 17))
                    for k in kb:
                        col = n * NCHUNK2 + k
                        ps = pss[k]
                        dst = rows(h1g1[n], MARG + (1 + 8 * k) * PW,
                                   8)[:, :, 1:57]
                        src = ps[:, 0:448].rearrange("p (r c) -> p r c", c=56)
                        nc.scalar.activation(dst, src, AF.Copy,
                                             accum_out=a1s[:, col:col + 1])
                        c1a_last[0] = nc.scalar.activation(
                            ps[:, 0:448].rearrange("p (r c) -> p r c", c=56),
                            dst, AF.Square, accum_out=a1q[:, col:col + 1])

            def conv1_og2():
                for n in range(NL):
                    for k in range(NCHUNK2):
                        # og2 partials (unshifted moving, all taps at once)
                        psp = psA.tile([128, 448], f32, tag="ps", name="ps")
                        off0 = MARG + STRIP0 + CHUNK2 * k
                        for cig in range(2):
                            rhs = x_pad[cig][n][:, off0:off0 + CHUNK2
                                                ].rearrange(
                                "p (r c) -> p r c", c=PW)[:, :, 0:56]
                            nc.tensor.matmul(
                                psp[0:IC2, :],
                                wt1b_sb[:, cig * IC2:(cig + 1) * IC2],
                                rhs, start=(cig == 0), stop=(cig == 1))
                        pdst = rows(pplane, MARG + (1 + 8 * k) * PW,
                                    8)[0:IC2, :, 1:57]
                        nc.scalar.activation(
                            pdst, psp[0:IC2, 0:448].rearrange(
                                "p (r c) -> p r c", c=56), AF.Copy)
                    for k in range(NCHUNK2):
                        col = n * NCHUNK2 + k
                        psc = psA.tile([128, 448], f32, tag="ps", name="ps")
                        for t, (ky, kx) in enumerate(_TAPS):
                            d = (ky - 1) * PW + (kx - 1)
                            off = MARG + STRIP0 + CHUNK2 * k + d
                            rhs = pplane[:, off:off + CHUNK2].rearrange(
                                "p (r c) -> p r c", c=PW)[:, :, 0:56]
                            nc.tensor.matmul(
                                psc[0:G2, :],
                                sel_sb[:, t * G2:(t + 1) * G2],
                                rhs, start=(t == 0), stop=(t == 8))
                        dst2 = rows(g2pack,
                                    n * PLANE_T + MARG + (1 + 8 * k) * PW,
                                    8)[0:G2, :, 1:57]
                        nc.scalar.activation(
                            dst2, psc[0:G2, 0:448].rearrange(
                                "p (r c) -> p r c", c=56), AF.Copy,
                            accum_out=a1s2[:, col:col + 1])
                        c1_last[0] = nc.scalar.activation(
                            psc[0:G2, 0:448].rearrange(
                                "p (r c) -> p r c", c=56),
                            dst2, AF.Square, accum_out=a1q2[:, col:col + 1])

            # ---- BN stats send/recv helpers (baseline plumbing) ----
            def bn_stats_send(reduces, ncols, ex):
                # one exchange of [128, ncols]; `reduces` is a list of
                # (acc_tile, P, col) pairs reduced into the packed tile.
                # Returns (rv, last DVE op) for recv pinning.
                packed = const.tile([128, 4], f32, tag=f"pk{ex}", name=f"pk{ex}")
                nc.vector.memset(packed[:], 0.0)
                for acc_t, P, colx in reduces:
                    nc.vector.tensor_reduce(
                        packed[0:P, colx:colx + 1], acc_t[:], axis=AX.X,
                        op=ALU.add)
                rv = const.tile([128, 8 * 4], f32, tag=f"rv{ex}", name=f"rv{ex}")
                cp = nc.vector.tensor_copy(rv[:, 0:ncols],
                                           packed[:, 0:ncols])   # own slot
                for d in range(1, 8):
                    rd = [None] * 8
                    rd[d] = (0, d)
                    gp_order(nc.gpsimd.remote_dma_broadcast(
                        rv[:, 4 * d:4 * d + ncols], packed[:, 0:ncols],
                        remote_sem=rsem[ex], local_sem=lsem, rdests=rd))
                gp_order(nc.gpsimd.trigger_dma(count=None))
                return rv, cp

            def bn_stats_recv(ex, rv, after, after_dve):
                gl = const.tile([128, 4], f32, tag=f"gl{ex}", name=f"gl{ex}")
                red = nc.vector.tensor_reduce(
                    gl[:], rv[:, 0:32].rearrange("p (s c) -> p c s", c=4),
                    axis=AX.X, op=ALU.add)
                defer_wait(red, rsem[ex], 14)
                bass_mod._add_dep_helper(red.ins, after.ins, sync=True,
                                         reason="recv after conv phase")
                bass_mod._add_dep_helper(red.ins, after_dve.ins, sync=False,
                                         reason="recv after own sends")
                return gl

            def bn_affine_finish(P, gl, gcol, g_ap, b_ap, sfx):
                mean = const.tile([P, 1], f32, tag=f"mean{sfx}", name=f"mean{sfx}")
                nc.vector.tensor_scalar_mul(mean[:], gl[0:P, gcol:gcol + 1], 1.0 / COUNT)
                var = const.tile([P, 1], f32, tag=f"var{sfx}", name=f"var{sfx}")
                nc.vector.tensor_tensor(var[:], mean[:], mean[:], ALU.mult)
                nc.vector.scalar_tensor_tensor(
                    var[:], gl[0:P, gcol + 1:gcol + 2], 1.0 / COUNT, var[:],
                    ALU.mult, ALU.subtract)
                nc.vector.tensor_scalar_add(var[:], var[:], EPS)
                y = const.tile([P, 1], f32, tag=f"y{sfx}", name=f"y{sfx}")
                vh = const.tile([P, 1], f32, tag=f"vh{sfx}", name=f"vh{sfx}")
                tmp = const.tile([P, 1], f32, tag=f"tm{sfx}", name=f"tm{sfx}")
                iv = var[:].bitcast(mybir.dt.int32)
                yi = y[:].bitcast(mybir.dt.int32)
                nc.vector.tensor_scalar(yi, iv, 1, None, ALU.arith_shift_right)
                nc.vector.tensor_scalar(yi, yi, -1, None, ALU.bitwise_xor)
                nc.vector.tensor_scalar(yi, yi, 0x5f3759df + 1, None, ALU.add)
                nc.vector.tensor_scalar_mul(vh[:], var[:], 0.5)
                for _ in range(2):
                    nc.vector.tensor_tensor(tmp[:], y[:], y[:], ALU.mult)
                    nc.vector.tensor_tensor(tmp[:], tmp[:], vh[:], ALU.mult)
                    nc.vector.tensor_scalar(tmp[:], tmp[:], -1.0, 1.5,
                                            ALU.mult, ALU.add)
                    nc.vector.tensor_tensor(y[:], y[:], tmp[:], ALU.mult)
                sc = const.tile([P, 1], f32, tag=f"sc{sfx}", name=f"sc{sfx}")
                nc.vector.tensor_tensor(sc[:], g_ap, y[:], ALU.mult)
                bi = const.tile([P, 1], f32, tag=f"bi{sfx}", name=f"bi{sfx}")
                nc.vector.tensor_tensor(bi[:], mean[:], sc[:], ALU.mult)
                nc.vector.tensor_tensor(bi[:], b_ap, bi[:], ALU.subtract)
                return sc, bi

            # ---- BN1 g1 stats fly while og2 computes ----
            rv1a, snd1a = bn_stats_send([(a1s, 128, 0), (a1q, 128, 1)], 2, 0)
            gl1a = bn_stats_recv(0, rv1a, c1a_last[0], snd1a)
            s1a, b1a = bn_affine_finish(128, gl1a, 0, aff_sb[:, 0:1],
                                        aff_sb[:, 2:3], "1a")
            conv1_og2()
            rv1b, snd1b = bn_stats_send([(a1s2, G2, 0), (a1q2, G2, 1)], 2, 1)
            gl1b = bn_stats_recv(1, rv1b, c1_last[0], snd1b)
            s1b, b1b = bn_affine_finish(G2, gl1b, 0, aff_sb[0:G2, 1:2],
                                        aff_sb[0:G2, 3:4], "1b")

            def g1_relu(n):
                for rh in range(2):
                    v = rows(h1g1[n], MARG + (1 + 28 * rh) * PW,
                             28)[:, :, 1:57]
                    r = nc.scalar.activation(v, v, AF.Relu,
                                             bias=b1a[:], scale=s1a[:])
                    # keep this arrival-gated ACT op behind all of the
                    # og2 phase's ACT stream (the scheduler's sim treats
                    # deferred waits as instant and would lift it earlier,
                    # stalling the in-order ACT queue)
                    bass_mod._add_dep_helper(r.ins, c1_last[0].ins,
                                             sync=False,
                                             reason="relu after og2 acts")
            for n in range(NL):
                # g2 relu on DVE so it never blocks the ACT stream
                v2 = rows(g2pack, n * PLANE_T + MARG + PW, 56)[0:G2, :, 1:57]
                nc.vector.tensor_scalar(v2, v2, s1b[:], None, ALU.mult)
                nc.vector.tensor_scalar(v2, v2, b1b[:], 0.0, ALU.add, ALU.max)
                for di, (ky, kx) in enumerate(_TAPS):
                    d = (ky - 1) * PW + (kx - 1)
                    nc.sync.dma_start(
                        im2col[di * G2:(di + 1) * G2,
                               n * IMLEN:(n + 1) * IMLEN],
                        g2pack[0:G2, n * PLANE_T + MARG + d:
                               n * PLANE_T + MARG + d + IMLEN])

            # ---- conv2: normal mode, home-layout output ----
            last_c2_act = [None]

            def conv2_B_img(j, n, act_after):
                last = act_after
                for k in range(NCHUNK2):
                    col = n * NCHUNK2 + k
                    psb = psA.tile([128, 448], f32, tag="ps", name="ps")
                    off = n * IMLEN + STRIP0 + CHUNK2 * k
                    rhs = im2col[:, off:off + CHUNK2].rearrange(
                        "p (r c) -> p r c", c=PW)[:, :, 0:56]
                    nc.tensor.matmul(
                        psb[:], wt2b_sb[:, j * 128:(j + 1) * 128], rhs,
                        start=True, stop=True)
                    h2f = h2[j][n][:, 8 * k * 56:(8 * k + 8) * 56]
                    nc.vector.tensor_tensor(h2f, h2f, psb[:, 0:448], ALU.add)
                    nc.vector.tensor_reduce(
                        a2[("s", j)][:, col:col + 1], h2f,
                        axis=AX.X, op=ALU.add)
                    sq = nc.scalar.activation(
                        psb[:, 0:448].rearrange("p (r c) -> p r c", c=56),
                        h2f.rearrange("p (r c) -> p r c", c=56),
                        AF.Square, accum_out=a2[("q", j)][:, col:col + 1])
                    if last is not None:
                        bass_mod._add_dep_helper(
                            sq.ins, last.ins, sync=False,
                            reason="B squares after prior acts")
                    last = sq
                return last

            def conv2_A(j, interleave_b=False, relu_cb=None):
                KB = (range(0, 4), range(4, 7))
                last = None
                for n in range(NL):
                    if relu_cb is not None:
                        relu_cb(n)
                    for kb in KB:
                        pss = {k: psA.tile([128, 448], f32, tag="ps",
                                           name="ps") for k in kb}
                        for t, (ky, kx) in enumerate(_TAPS):
                            d = (ky - 1) * PW + (kx - 1)
                            for k in kb:
                                off = MARG + STRIP0 + CHUNK2 * k + d
                                rhs = h1g1[n][:, off:off + CHUNK2].rearrange(
                                    "p (r c) -> p r c", c=PW)[:, :, 0:56]
                                nc.tensor.matmul(
                                    pss[k][:],
                                    wt2a_sb[:, (j * 9 + t) * 128:
                                            (j * 9 + t + 1) * 128],
                                    rhs, start=(t == 0), stop=(t == 8))
                        for k in kb:
                            dst_int = h2[j][n][:, 8 * k * 56:(8 * k + 8) * 56
                                               ].rearrange(
                                "p (r c) -> p r c", c=56)
                            last = nc.scalar.activation(
                                dst_int, pss[k][:, 0:448].rearrange(
                                    "p (r c) -> p r c", c=56), AF.Copy)
                    # B phases of BOTH halves ride two images behind A
                    # (their im2col inputs are arrival-gated; the lag keeps
                    # the in-order ACT queue from stalling on late stats)
                    if interleave_b and n >= 1:
                        last = conv2_B_img(1, n - 1, last)
                if interleave_b:
                    last = conv2_B_img(1, NL - 1, last)
                return last

            def make_tail(j, s2, b2, use_act=True, alternate=False):
                # out = relu(s2*h2 + b2 + x); x from resident bf16 planes.
                # alternate=True spreads the element work over three engines
                # (DVE / GPSIMD for the fused multiply-add, ACT / DVE for the
                # relu) since the tail is the only work running at this point
                def mk(n, rh):
                    def emit():
                        m = n * NHALF + rh
                        r0 = rh * HALF_ROWS
                        xv = rows(x_pad[j][n], MARG + (r0 + 1) * PW,
                                  HALF_ROWS)[:, :, 1:57]
                        h2v = h2[j][n][:, r0 * W:r0 * W + HALF_ELEMS
                                       ].rearrange("p (r c) -> p r c", c=W)
                        ot = xst.tile([128, HALF_ELEMS], f32, tag="xs",
                                      name="ot")
                        otv = ot[:, :].rearrange("p (r c) -> p r c", c=W)
                        nc.vector.scalar_tensor_tensor(
                            otv, h2v, s2[:], xv, ALU.mult, ALU.add)
                        if alternate and m % 2 == 1:
                            nc.vector.tensor_scalar(ot[:], ot[:], b2[:], 0.0,
                                                    ALU.add, ALU.max)
                        else:
                            r = nc.scalar.activation(ot[:], ot[:], AF.Relu,
                                                     bias=b2[:], scale=1.0)
                            bass_mod._add_dep_helper(
                                r.ins, last_c2_act[0].ins, sync=False,
                                reason="tail relu after conv2 acts")
                        nc.sync.dma_start(
                            out_d[n, j * 128:(j + 1) * 128,
                                  r0:r0 + HALF_ROWS, :],
                            ot[:])
                    return emit
                return [mk(n, rh) for n in range(NL) for rh in range(NHALF)]

            a0_last = conv2_A(0, relu_cb=g1_relu)
            c2h1_last = conv2_A(1, interleave_b=True)
            rv2b, snd2b = bn_stats_send(
                [(a2[("s", 1)], 128, 0), (a2[("q", 1)], 128, 1)], 2, 2)
            c2h0_last = c2h1_last
            for n in range(NL):
                c2h0_last = conv2_B_img(0, n, c2h0_last)
            last_c2_act[0] = c2h0_last
            rv2a, snd2a = bn_stats_send(
                [(a2[("s", 0)], 128, 0), (a2[("q", 0)], 128, 1)], 2, 3)
            # half-1 stats arrive while half-0's B phase runs
            gl2b = bn_stats_recv(2, rv2b, c2h1_last, snd2b)
            s2b, b2b = bn_affine_finish(128, gl2b, 0, aff_sb[:, 5:6],
                                        aff_sb[:, 7:8], "2b")
            for blk in make_tail(1, s2b, b2b):
                blk()
            gl2a = bn_stats_recv(3, rv2a, c2h0_last, snd2a)
            s2a, b2a = bn_affine_finish(128, gl2a, 0, aff_sb[:, 4:5],
                                        aff_sb[:, 6:7], "2a")
            for blk in make_tail(0, s2a, b2a):
                blk()

    for bi, sem, val in deferred_waits:
        patched = False
        for w in bi.ins.sync_info.on_wait:
            if w.id == sem.num and w.wait_value == 0:
                w.wait_value = val
                patched = True
                break
        assert patched, f"deferred wait not found on {bi.ins.name}"

    nc.compile()
    return nc


def kernel(x, W1, W2, gamma1, beta1, gamma2, beta2, mask1, mask2,
           _trace=False, _trace_kwargs=None):
    from concourse.bass_utils import run_bass_kernel_spmd

    K1, wt1a, wt1b, sel, wt2a, wt2b, aff = _pack(
        W1, W2, mask1, mask2, gamma1, beta1, gamma2, beta2)

    key = ("v2", K1)
    if key not in _cache:
        _cache[key] = _build(K1)
    nc = _cache[key]

    x = np.ascontiguousarray(np.asarray(x, np.float32))
    in_maps = [{"x": x[i * NL:(i + 1) * NL], "wt1a": wt1a, "wt1b": wt1b,
                "sel": sel, "wt2a": wt2a, "wt2b": wt2b, "aff": aff}
               for i in range(N_CORES)]
    kw = {}
    if _trace:
        kw = dict(trace=True, **(_trace_kwargs or {}))
    res = run_bass_kernel_spmd(nc, in_maps, core_ids=list(range(N_CORES)), **kw)
    out = np.concatenate([res.results[i]["out"] for i in range(N_CORES)], axis=0)
    _cache["last_results"] = res
    return out


# revision 31
# speedup vs baseline: 1.1317x; 1.1317x over previous
"""Trainium2 Bass kernel for a ResNet BasicBlock (dense CNN, sync-BN).

Reference computation (training-mode BN, batch stats over (N,H,W)):
    h = conv3x3(x, W1) * mask1            # structured channel pruning
    h = relu(bn(h, gamma1, beta1))
    h = conv3x3(h, W2) * mask2
    h = bn(h, gamma2, beta2)
    out = relu(h + x)                      # identity shortcut

Shapes: x [32, 256, 56, 56] f32, W [256, 256, 3, 3] f32.

Strategy: data-parallel over batch N across 8 NeuronCores (4 images per
core), weights replicated, BN stats synchronized with small remote-DMA
all-broadcasts of per-channel (sum, sumsq) pairs.

Mask specialization (compiled per mask pattern): only the K1 = |mask1|
live conv1 channels are computed, gathered as a 128-channel group (og1)
plus a G2 = K1-128 overflow group (og2).
  - og1: normal-mode conv, tap-outer over 4-chunk PSUM blocks so each
    LDWEIGHTS amortizes over 4 matmuls.
  - og2 (dual im2col): tap-partials P[(t,c),q] = sum_ci x[ci,q]*W1 for
    all 9*G2 rows in 2 matmuls/chunk (unshifted moving operand), then
    9 shifted selector-matmuls combine them -- 11 MM/chunk vs 18.
  - conv2: home-layout output (masked output channels get zero weights,
    so BN2/tail are mask-oblivious); contraction over the live h1
    channels as one 128-row group (9 tap matmuls, phase A) plus one
    9*G2-row im2col group (1 matmul, phase B; shifted copies built by
    9 SBUF->SBUF DMAs per image), 10 MM/chunk vs 18.
Scheduling: og1 stats are exchanged while og2 computes; g2 stats fly
during conv2 phase A; conv2's phase B for half 1 is interleaved one
image behind phase A so the half-1 exchange fires right at A's end, and
half 0's phase B runs while those stats are in flight.  Arrival-gated
ACT/DVE ops carry explicit order-pins so the in-order engine queues
never stall behind a blocked op (the scheduler's sim treats deferred
remote-semaphore waits as instantly satisfied).  The one exposed
synchronization is the final half-0 exchange, which absorbs the
cross-core launch skew.
"""

import numpy as np
import ml_dtypes

# ---- problem constants (hardcoded; kernel.py must be self-contained) ----
N_TOT, C, H, W = 32, 256, 56, 56
N_CORES = 8
NL = N_TOT // N_CORES          # images per core
PW = H + 2                     # padded row stride (58)
MARG = 64                      # front/back zero margin for shifted reads
PPLANE = PW * PW               # 3364 padded positions
PLANE_T = MARG + PPLANE + MARG # full plane tile length (3492)
STRIP0 = PW + 1                # first interior position (59), plane-relative
CHUNK2 = 8 * PW                # 464: conv2 processes 8 output rows per chunk
NCHUNK2 = 7
NCHUNK1 = 28                   # conv1-T: 28 chunks x 2 rows
C1P = 2 * PW                   # 116 positions per conv1-T chunk
IMLEN = 3368                   # im2col valid length (conv2 reads q in [59,3307))
HW = H * W                     # 3136
HALF_ROWS = 14                 # row granularity for x/out streaming DMAs
HALF_ELEMS = HALF_ROWS * W
NHALF = H // HALF_ROWS         # 4
COUNT = N_TOT * HW             # sync-BN element count per channel
EPS = 1e-5

_BF16 = ml_dtypes.bfloat16
_TAPS = [(ky, kx) for ky in range(3) for kx in range(3)]

_cache = {}


def _pack(W1, W2, mask1, mask2, gamma1, beta1, gamma2, beta2):
    S1 = np.nonzero(np.asarray(mask1) != 0)[0]
    K1 = len(S1)
    assert 128 < K1 <= 256, f"kernel specialized for 128<K1<=256, got {K1}"
    G2 = K1 - 128
    g1, g2 = S1[:128], S1[128:]

    W1 = np.asarray(W1, np.float32)
    W2m = (np.asarray(W2, np.float32)
           * np.asarray(mask2, np.float32)[:, None, None, None])

    # conv1 stationary tiles: og1 [ci, (cig,tap) x g1-chans],
    # og2 [ci, (cig,tap) x g2-chans]  (normal mode, gathered outputs)
    wt1a = np.empty((128, 18 * 128), np.float32)
    for cig in range(2):
        for t, (ky, kx) in enumerate(_TAPS):
            blk = cig * 9 + t
            wt1a[:, blk * 128:(blk + 1) * 128] = \
                W1[g1, cig * 128:(cig + 1) * 128, ky, kx].T
    # og2 dual: tap-partials P[(t,c),q] = sum_ci x[ci,q] W1[g2[c],ci,t];
    # combined later with shifted selector matmuls
    wt1b = np.empty((128, 2 * 9 * G2), np.float32)
    for cig in range(2):
        for t, (ky, kx) in enumerate(_TAPS):
            wt1b[:, cig * 9 * G2 + t * G2:cig * 9 * G2 + (t + 1) * G2] = \
                W1[g2, cig * 128:(cig + 1) * 128, ky, kx].T
    sel = np.eye(9 * G2, dtype=np.float32)

    # conv2 stationary, g1 group: [ci=h1 partition i (ch g1[i]), co home]
    wt2a = np.empty((128, 18 * 128), np.float32)
    for j in range(2):
        for t, (ky, kx) in enumerate(_TAPS):
            blk = j * 9 + t
            wt2a[:, blk * 128:(blk + 1) * 128] = \
                W2m[j * 128:(j + 1) * 128, :, ky, kx][:, g1].T

    # conv2 stationary, im2col group: row di*G2+c -> (tap di, ch g2[c])
    wt2b = np.empty((9 * G2, 2 * 128), np.float32)
    for j in range(2):
        for di, (ky, kx) in enumerate(_TAPS):
            wt2b[di * G2:(di + 1) * G2, j * 128:(j + 1) * 128] = \
                W2m[j * 128:(j + 1) * 128, :, ky, kx][:, g2].T

    # BN affine params: BN1 in gathered order, BN2 in home order
    aff = np.zeros((128, 8), np.float32)
    aff[:, 0] = np.asarray(gamma1, np.float32)[g1]
    aff[:G2, 1] = np.asarray(gamma1, np.float32)[g2]
    aff[:, 2] = np.asarray(beta1, np.float32)[g1]
    aff[:G2, 3] = np.asarray(beta1, np.float32)[g2]
    aff[:, 4] = np.asarray(gamma2, np.float32).reshape(2, 128)[0]
    aff[:, 5] = np.asarray(gamma2, np.float32).reshape(2, 128)[1]
    aff[:, 6] = np.asarray(beta2, np.float32).reshape(2, 128)[0]
    aff[:, 7] = np.asarray(beta2, np.float32).reshape(2, 128)[1]

    return (K1, wt1a.astype(_BF16), wt1b.astype(_BF16), sel.astype(_BF16),
            wt2a.astype(_BF16), wt2b.astype(_BF16), aff)


def _build(K1):
    import concourse.bass as bass_mod
    import concourse.bacc as bacc
    import concourse.mybir as mybir
    import concourse.tile as tile

    G2 = K1 - 128
    IC2 = 9 * G2

    f32 = mybir.dt.float32
    bf16 = mybir.dt.bfloat16
    AX = mybir.AxisListType
    ALU = mybir.AluOpType
    AF = mybir.ActivationFunctionType

    nc = bacc.Bacc("TRN2", target_bir_lowering=False, debug=False,
                   num_devices=N_CORES)

    x_d = nc.dram_tensor("x", [NL, C, H, W], f32, kind="ExternalInput")
    wt1a_d = nc.dram_tensor("wt1a", [128, 18 * 128], bf16, kind="ExternalInput")
    wt1b_d = nc.dram_tensor("wt1b", [128, 2 * 9 * G2], bf16, kind="ExternalInput")
    sel_d = nc.dram_tensor("sel", [IC2, IC2], bf16, kind="ExternalInput")
    wt2a_d = nc.dram_tensor("wt2a", [128, 18 * 128], bf16, kind="ExternalInput")
    wt2b_d = nc.dram_tensor("wt2b", [IC2, 2 * 128], bf16, kind="ExternalInput")
    aff_d = nc.dram_tensor("aff", [128, 8], f32, kind="ExternalInput")
    out_d = nc.dram_tensor("out", [NL, C, H, W], f32, kind="ExternalOutput")

    groups = [list(range(N_CORES))]

    def rows(tile_ap, base, nrows, off=0):
        """[P, nrows, 58] view of whole padded rows starting at plane
        offset `base` (row-aligned, absolute incl. margin)."""
        v = tile_ap[:, base + off:base + off + nrows * PW].rearrange(
            "p (r c) -> p r c", c=PW)
        return v

    with tile.TileContext(nc) as tc:
        import contextlib
        with contextlib.ExitStack() as ctx:
            const = ctx.enter_context(tc.tile_pool(name="const", bufs=1))
            psA = ctx.enter_context(tc.tile_pool(name="psA", bufs=6, space="PSUM"))
            xst = ctx.enter_context(tc.tile_pool(name="xst", bufs=3))

            wt1a_sb = const.tile([128, 18 * 128], bf16, tag="wt1a", name="wt1a")
            nc.sync.dma_start(wt1a_sb[:], wt1a_d[:])
            wt1b_sb = const.tile([128, 2 * 9 * G2], bf16, tag="wt1b", name="wt1b")
            nc.sync.dma_start(wt1b_sb[:], wt1b_d[:])
            sel_sb = const.tile([IC2, IC2], bf16, tag="sel", name="sel")
            nc.sync.dma_start(sel_sb[:], sel_d[:])
            wt2a_sb = const.tile([128, 18 * 128], bf16, tag="wt2a", name="wt2a")
            nc.sync.dma_start(wt2a_sb[:], wt2a_d[:])
            wt2b_sb = const.tile([IC2, 2 * 128], bf16, tag="wt2b", name="wt2b")
            nc.sync.dma_start(wt2b_sb[:], wt2b_d[:])
            aff_sb = const.tile([128, 8], f32, tag="aff", name="aff")
            nc.sync.dma_start(aff_sb[:], aff_d[:])

            # ---- cross-core stats exchange plumbing (SBUF remote DMA) ----
            rsem = [nc.alloc_semaphore(f"rst{i}") for i in range(4)]
            lsem = nc.alloc_semaphore("lst")
            _gp_prev = [None]
            deferred_waits = []

            def gp_order(bi):
                if _gp_prev[0] is not None:
                    bass_mod._add_dep_helper(bi.ins, _gp_prev[0].ins,
                                             sync=False,
                                             reason="stats-exchange order")
                _gp_prev[0] = bi
                return bi

            nc._bir_kernel_barrier_sem_replica_groups.extend(
                set(g) for g in groups)

            def defer_wait(bi, sem, val):
                bi._wait_ge(sem, 0)
                deferred_waits.append((bi, sem, val))
                return bi

            for i, s in enumerate(rsem + [lsem]):
                cl = gp_order(nc.gpsimd.sem_clear(s))
                if i == 0:
                    defer_wait(cl, nc._bir_kernel_barrier_sem,
                               nc.bir_kernel_barrier_sem_inc)

            # persistent per-image planes
            x_pad = [[const.tile([128, PLANE_T], bf16, tag=f"xp{j}_{n}",
                                 name=f"xp{j}_{n}")
                      for n in range(NL)] for j in range(2)]
            h1g1 = [const.tile([128, PLANE_T], bf16, tag=f"h1_{n}",
                               name=f"h1_{n}") for n in range(NL)]
            g2pack = const.tile([G2, NL * PLANE_T], bf16, tag="g2p", name="g2p")
            im2col = const.tile([IC2, 2 * IMLEN], bf16, tag="i2c", name="i2c")
            h2 = [[const.tile([128, HW], bf16, tag=f"h2{j}_{n}",
                              name=f"h2{j}_{n}")
                   for n in range(NL)] for j in range(2)]

            # zero only the non-interior plane structure (front margin +
            # top pad row, the 56 pad-column pairs, bottom pad row + back
            # margin); x planes of image 0 first so conv1 starts early
            def zero_pads(t, base, P=128):
                nc.vector.memset(t[0:P, base:base + MARG + STRIP0], 0.0)
                pairs = t[0:P, base + MARG + 2 * PW - 1:
                          base + MARG + 2 * PW - 1 + 56 * PW].rearrange(
                    "p (r c) -> p r c", c=PW)[:, :, 0:2]
                nc.vector.memset(pairs, 0.0)
                nc.vector.memset(
                    t[0:P, base + MARG + STRIP0 + 56 * PW:base + PLANE_T], 0.0)
            def head_img(n):
                for j in range(2):
                    for rh in range(NHALF):
                        r0 = rh * HALF_ROWS
                        xs = xst.tile([128, HALF_ELEMS], f32, tag="xs",
                                      name="xs")
                        nc.sync.dma_start(
                            xs[:],
                            x_d[n, j * 128:(j + 1) * 128,
                                r0:r0 + HALF_ROWS, :])
                        dst = rows(x_pad[j][n], MARG + (r0 + 1) * PW,
                                   HALF_ROWS)[:, :, 1:57]
                        src = xs[:, :].rearrange("p (r c) -> p r c", c=W)
                        nc.vector.tensor_copy(dst, src)
            for j in range(2):
                zero_pads(x_pad[j][0], 0)
            head_img(0)
            for n in range(1, NL):
                for j in range(2):
                    zero_pads(x_pad[j][n], 0)
            for n in range(NL):
                zero_pads(h1g1[n], 0)
                zero_pads(g2pack, n * PLANE_T, P=G2)
            zero_pads(pplane, 0, P=IC2)

            # stats accumulators: one column per (image, chunk)
            a1s = const.tile([128, NL * NCHUNK2], f32, tag="a1s", name="a1s")
            a1q = const.tile([128, NL * NCHUNK2], f32, tag="a1q", name="a1q")
            a1s2 = const.tile([G2, NL * NCHUNK2], f32, tag="a1s2", name="a1s2")
            a1q2 = const.tile([G2, NL * NCHUNK2], f32, tag="a1q2", name="a1q2")
            a2 = {(s, j): const.tile([128, NL * NCHUNK2], f32,
                                     tag=f"a2{s}{j}", name=f"a2{s}{j}")
                  for s in ("s", "q") for j in range(2)}

            # ---- head: stream the remaining images in ----
            for n in range(1, NL):
                head_img(n)

            # ---- conv1: og1 direct (18 MM/chunk); og2 via dual im2col
            # (2 partial MMs + 9 shifted selector MMs per chunk) ----
            c1_last = [None]
            c1a_last = [None]
            KB = (range(0, 4), range(4, 7))
            # og1 phase (all images) -- its stats are exchanged while the
            # og2 phase still computes, hiding one sync round-trip
            for n in range(NL):
                for kb in KB:
                    pss = {k: psA.tile([128, 448], f32, tag="ps", name="ps")
                           for k in kb}
                    for idx, (cig, (t, (ky, kx))) in enumerate(
                            (c, tt) for c in range(2)
                            for tt in enumerate(_TAPS)):
                        d = (ky - 1) * PW + (kx - 1)
                        blk = cig * 9 + t
                        for k in kb:
                            off = MARG + STRIP0 + CHUNK2 * k + d
                            rhs = x_pad[cig][n][:, off:off + CHUNK2
                                                ].rearrange(
                                "p (r c) -> p r c", c=PW)[:, :, 0:56]
                            nc.tensor.matmul(
                                pss[k][0:128, :],
                                wt1a_sb[:, blk * 128:(blk + 1) * 128],
                                rhs, start=(idx == 0), stop=(idx == 17))
                    for k in kb:
                        col = n * NCHUNK2 + k
                        ps = pss[k]
                        dst = rows(h1g1[n], MARG + (1 + 8 * k) * PW,
                                   8)[:, :, 1:57]
                        src = ps[:, 0:448].rearrange("p (r c) -> p r c", c=56)
                        nc.scalar.activation(dst, src, AF.Copy,
                                             accum_out=a1s[:, col:col + 1])
                        c1a_last[0] = nc.scalar.activation(
                            ps[:, 0:448].rearrange("p (r c) -> p r c", c=56),
                            dst, AF.Square, accum_out=a1q[:, col:col + 1])

            def conv1_og2():
                for n in range(NL):
                    for k in range(NCHUNK2):
                        # og2 partials (unshifted moving, all taps at once)
                        psp = psA.tile([128, 448], f32, tag="ps", name="ps")
                        off0 = MARG + STRIP0 + CHUNK2 * k
                        for cig in range(2):
                            rhs = x_pad[cig][n][:, off0:off0 + CHUNK2
                                                ].rearrange(
                                "p (r c) -> p r c", c=PW)[:, :, 0:56]
                            nc.tensor.matmul(
                                psp[0:IC2, :],
                                wt1b_sb[:, cig * IC2:(cig + 1) * IC2],
                                rhs, start=(cig == 0), stop=(cig == 1))
                        pdst = rows(pplane, MARG + (1 + 8 * k) * PW,
                                    8)[0:IC2, :, 1:57]
                        nc.scalar.activation(
                            pdst, psp[0:IC2, 0:448].rearrange(
                                "p (r c) -> p r c", c=56), AF.Copy)
                    for k in range(NCHUNK2):
                        col = n * NCHUNK2 + k
                        psc = psA.tile([128, 448], f32, tag="ps", name="ps")
                        for t, (ky, kx) in enumerate(_TAPS):
                            d = (ky - 1) * PW + (kx - 1)
                            off = MARG + STRIP0 + CHUNK2 * k + d
                            rhs = pplane[:, off:off + CHUNK2].rearrange(
                                "p (r c) -> p r c", c=PW)[:, :, 0:56]
                            nc.tensor.matmul(
                                psc[0:G2, :],
                                sel_sb[:, t * G2:(t + 1) * G2],
                                rhs, start=(t == 0), stop=(t == 8))
                        dst2 = rows(g2pack,
                                    n * PLANE_T + MARG + (1 + 8 * k) * PW,
                                    8)[0:G2, :, 1:57]
                        nc.scalar.activation(
                            dst2, psc[0:G2, 0:448].rearrange(
                                "p (r c) -> p r c", c=56), AF.Copy,
                            accum_out=a1s2[:, col:col + 1])
                        c1_last[0] = nc.scalar.activation(
                            psc[0:G2, 0:448].rearrange(
                                "p (r c) -> p r c", c=56),
                            dst2, AF.Square, accum_out=a1q2[:, col:col + 1])

            # ---- BN stats send/recv helpers (baseline plumbing) ----
            def bn_stats_send(reduces, ncols, ex):
                # one exchange of [128, ncols]; `reduces` is a list of
                # (acc_tile, P, col) pairs reduced into the packed tile.
                # Returns (rv, last DVE op) for recv pinning.
                packed = const.tile([128, 4], f32, tag=f"pk{ex}", name=f"pk{ex}")
                nc.vector.memset(packed[:], 0.0)
                for acc_t, P, colx in reduces:
                    nc.vector.tensor_reduce(
                        packed[0:P, colx:colx + 1], acc_t[:], axis=AX.X,
                        op=ALU.add)
                rv = const.tile([128, 8 * 4], f32, tag=f"rv{ex}", name=f"rv{ex}")
                cp = nc.vector.tensor_copy(rv[:, 0:ncols],
                                           packed[:, 0:ncols])   # own slot
                for d in range(1, 8):
                    rd = [None] * 8
                    rd[d] = (0, d)
                    gp_order(nc.gpsimd.remote_dma_broadcast(
                        rv[:, 4 * d:4 * d + ncols], packed[:, 0:ncols],
                        remote_sem=rsem[ex], local_sem=lsem, rdests=rd))
                gp_order(nc.gpsimd.trigger_dma(count=None))
                return rv, cp

            def bn_stats_recv(ex, rv, after, after_dve):
                gl = const.tile([128, 4], f32, tag=f"gl{ex}", name=f"gl{ex}")
                red = nc.vector.tensor_reduce(
                    gl[:], rv[:, 0:32].rearrange("p (s c) -> p c s", c=4),
                    axis=AX.X, op=ALU.add)
                defer_wait(red, rsem[ex], 14)
                bass_mod._add_dep_helper(red.ins, after.ins, sync=True,
                                         reason="recv after conv phase")
                bass_mod._add_dep_helper(red.ins, after_dve.ins, sync=False,
                                         reason="recv after own sends")
                return gl

            def bn_affine_finish(P, gl, gcol, g_ap, b_ap, sfx):
                mean = const.tile([P, 1], f32, tag=f"mean{sfx}", name=f"mean{sfx}")
                nc.vector.tensor_scalar_mul(mean[:], gl[0:P, gcol:gcol + 1], 1.0 / COUNT)
                var = const.tile([P, 1], f32, tag=f"var{sfx}", name=f"var{sfx}")
                nc.vector.tensor_tensor(var[:], mean[:], mean[:], ALU.mult)
                nc.vector.scalar_tensor_tensor(
                    var[:], gl[0:P, gcol + 1:gcol + 2], 1.0 / COUNT, var[:],
                    ALU.mult, ALU.subtract)
                nc.vector.tensor_scalar_add(var[:], var[:], EPS)
                y = const.tile([P, 1], f32, tag=f"y{sfx}", name=f"y{sfx}")
                vh = const.tile([P, 1], f32, tag=f"vh{sfx}", name=f"vh{sfx}")
                tmp = const.tile([P, 1], f32, tag=f"tm{sfx}", name=f"tm{sfx}")
                iv = var[:].bitcast(mybir.dt.int32)
                yi = y[:].bitcast(mybir.dt.int32)
                nc.vector.tensor_scalar(yi, iv, 1, None, ALU.arith_shift_right)
                nc.vector.tensor_scalar(yi, yi, -1, None, ALU.bitwise_xor)
                nc.vector.tensor_scalar(yi, yi, 0x5f3759df + 1, None, ALU.add)
                nc.vector.tensor_scalar_mul(vh[:], var[:], 0.5)
                for _ in range(2):
                    nc.vector.tensor_tensor(tmp[:], y[:], y[:], ALU.mult)
                    nc.vector.tensor_tensor(tmp[:], tmp[:], vh[:], ALU.mult)
                    nc.vector.tensor_scalar(tmp[:], tmp[:], -1.0, 1.5,
                                            ALU.mult, ALU.add)
                    nc.vector.tensor_tensor(y[:], y[:], tmp[:], ALU.mult)
                sc = const.tile([P, 1], f32, tag=f"sc{sfx}", name=f"sc{sfx}")
                nc.vector.tensor_tensor(sc[:], g_ap, y[:], ALU.mult)
                bi = const.tile([P, 1], f32, tag=f"bi{sfx}", name=f"bi{sfx}")
                nc.vector.tensor_tensor(bi[:], mean[:], sc[:], ALU.mult)
                nc.vector.tensor_tensor(bi[:], b_ap, bi[:], ALU.subtract)
                return sc, bi

            # ---- BN1 g1 stats fly while og2 computes ----
            rv1a, snd1a = bn_stats_send([(a1s, 128, 0), (a1q, 128, 1)], 2, 0)
            gl1a = bn_stats_recv(0, rv1a, c1a_last[0], snd1a)
            s1a, b1a = bn_affine_finish(128, gl1a, 0, aff_sb[:, 0:1],
                                        aff_sb[:, 2:3], "1a")
            conv1_og2()
            rv1b, snd1b = bn_stats_send([(a1s2, G2, 0), (a1q2, G2, 1)], 2, 1)
            gl1b = bn_stats_recv(1, rv1b, c1_last[0], snd1b)
            s1b, b1b = bn_affine_finish(G2, gl1b, 0, aff_sb[0:G2, 1:2],
                                        aff_sb[0:G2, 3:4], "1b")

            def g1_relu(n):
                for rh in range(2):
                    v = rows(h1g1[n], MARG + (1 + 28 * rh) * PW,
                             28)[:, :, 1:57]
                    r = nc.scalar.activation(v, v, AF.Relu,
                                             bias=b1a[:], scale=s1a[:])
                    # keep this arrival-gated ACT op behind all of the
                    # og2 phase's ACT stream (the scheduler's sim treats
                    # deferred waits as instant and would lift it earlier,
                    # stalling the in-order ACT queue)
                    bass_mod._add_dep_helper(r.ins, c1_last[0].ins,
                                             sync=False,
                                             reason="relu after og2 acts")
            for n in range(NL):
                # g2 relu on DVE so it never blocks the ACT stream
                v2 = rows(g2pack, n * PLANE_T + MARG + PW, 56)[0:G2, :, 1:57]
                nc.vector.tensor_scalar(v2, v2, s1b[:], None, ALU.mult)
                nc.vector.tensor_scalar(v2, v2, b1b[:], 0.0, ALU.add, ALU.max)
                for di, (ky, kx) in enumerate(_TAPS):
                    d = (ky - 1) * PW + (kx - 1)
                    nc.sync.dma_start(
                        im2col[di * G2:(di + 1) * G2,
                               n * IMLEN:(n + 1) * IMLEN],
                        g2pack[0:G2, n * PLANE_T + MARG + d:
                               n * PLANE_T + MARG + d + IMLEN])

            # ---- conv2: normal mode, home-layout output ----
            last_c2_act = [None]

            def conv2_B_img(j, n, act_after):
                last = act_after
                for k in range(NCHUNK2):
                    col = n * NCHUNK2 + k
                    psb = psA.tile([128, 448], f32, tag="ps", name="ps")
                    off = n * IMLEN + STRIP0 + CHUNK2 * k
                    rhs = im2col[:, off:off + CHUNK2].rearrange(
                        "p (r c) -> p r c", c=PW)[:, :, 0:56]
                    nc.tensor.matmul(
                        psb[:], wt2b_sb[:, j * 128:(j + 1) * 128], rhs,
                        start=True, stop=True)
                    h2f = h2[j][n][:, 8 * k * 56:(8 * k + 8) * 56]
                    nc.vector.tensor_tensor(h2f, h2f, psb[:, 0:448], ALU.add)
                    nc.vector.tensor_reduce(
                        a2[("s", j)][:, col:col + 1], h2f,
                        axis=AX.X, op=ALU.add)
                    sq = nc.scalar.activation(
                        psb[:, 0:448].rearrange("p (r c) -> p r c", c=56),
                        h2f.rearrange("p (r c) -> p r c", c=56),
                        AF.Square, accum_out=a2[("q", j)][:, col:col + 1])
                    if last is not None:
                        bass_mod._add_dep_helper(
                            sq.ins, last.ins, sync=False,
                            reason="B squares after prior acts")
                    last = sq
                return last

            def conv2_A(j, interleave_b=False, relu_cb=None):
                KB = (range(0, 4), range(4, 7))
                last = None
                for n in range(NL):
                    if relu_cb is not None:
                        relu_cb(n)
                    for kb in KB:
                        pss = {k: psA.tile([128, 448], f32, tag="ps",
                                           name="ps") for k in kb}
                        for t, (ky, kx) in enumerate(_TAPS):
                            d = (ky - 1) * PW + (kx - 1)
                            for k in kb:
                                off = MARG + STRIP0 + CHUNK2 * k + d
                                rhs = h1g1[n][:, off:off + CHUNK2].rearrange(
                                    "p (r c) -> p r c", c=PW)[:, :, 0:56]
                                nc.tensor.matmul(
                                    pss[k][:],
                                    wt2a_sb[:, (j * 9 + t) * 128:
                                            (j * 9 + t + 1) * 128],
                                    rhs, start=(t == 0), stop=(t == 8))
                        for k in kb:
                            dst_int = h2[j][n][:, 8 * k * 56:(8 * k + 8) * 56
                                               ].rearrange(
                                "p (r c) -> p r c", c=56)
                            last = nc.scalar.activation(
                                dst_int, pss[k][:, 0:448].rearrange(
                                    "p (r c) -> p r c", c=56), AF.Copy)
                    # B phases of BOTH halves ride two images behind A
                    # (their im2col inputs are arrival-gated; the lag keeps
                    # the in-order ACT queue from stalling on late stats)
                    if interleave_b and n >= 1:
                        last = conv2_B_img(1, n - 1, last)
                if interleave_b:
                    last = conv2_B_img(1, NL - 1, last)
                return last

            def make_tail(j, s2, b2, use_act=True, alternate=False):
                # out = relu(s2*h2 + b2 + x); x from resident bf16 planes.
                # alternate=True spreads the element work over three engines
                # (DVE / GPSIMD for the fused multiply-add, ACT / DVE for the
                # relu) since the tail is the only work running at this point
                def mk(n, rh):
                    def emit():
                        m = n * NHALF + rh
                        r0 = rh * HALF_ROWS
                        xv = rows(x_pad[j][n], MARG + (r0 + 1) * PW,
                                  HALF_ROWS)[:, :, 1:57]
                        h2v = h2[j][n][:, r0 * W:r0 * W + HALF_ELEMS
                                       ].rearrange("p (r c) -> p r c", c=W)
                        ot = xst.tile([128, HALF_ELEMS], f32, tag="xs",
                                      name="ot")
                        otv = ot[:, :].rearrange("p (r c) -> p r c", c=W)
                        nc.vector.scalar_tensor_tensor(
                            otv, h2v, s2[:], xv, ALU.mult, ALU.add)
                        if alternate and m % 2 == 1:
                            nc.vector.tensor_scalar(ot[:], ot[:], b2[:], 0.0,
                                                    ALU.add, ALU.max)
                        else:
                            r = nc.scalar.activation(ot[:], ot[:], AF.Relu,
                                                     bias=b2[:], scale=1.0)
                            bass_mod._add_dep_helper(
                                r.ins, last_c2_act[0].ins, sync=False,
                                reason="tail relu after conv2 acts")
                        nc.sync.dma_start(
                            out_d[n, j * 128:(j + 1) * 128,
                                  r0:r0 + HALF_ROWS, :],
                            ot[:])
                    return emit
                return [mk(n, rh) for n in range(NL) for rh in range(NHALF)]

            a0_last = conv2_A(0, relu_cb=g1_relu)
            c2h1_last = conv2_A(1, interleave_b=True)
            rv2b, snd2b = bn_stats_send(
                [(a2[("s", 1)], 128, 0), (a2[("q", 1)], 128, 1)], 2, 2)
            c2h0_last = c2h1_last
            for n in range(NL):
                c2h0_last = conv2_B_img(0, n, c2h0_last)
            last_c2_act[0] = c2h0_last
            rv2a, snd2a = bn_stats_send(
                [(a2[("s", 0)], 128, 0), (a2[("q", 0)], 128, 1)], 2, 3)
            # half-1 stats arrive while half-0's B phase runs
            gl2b = bn_stats_recv(2, rv2b, c2h1_last, snd2b)
            s2b, b2b = bn_affine_finish(128, gl2b, 0, aff_sb[:, 5:6],
                                        aff_sb[:, 7:8], "2b")
            for blk in make_tail(1, s2b, b2b):
                blk()
            gl2a = bn_stats_recv(3, rv2a, c2h0_last, snd2a)
            s2a, b2a = bn_affine_finish(128, gl2a, 0, aff_sb[:, 4:5],
                                        aff_sb[:, 6:7], "2a")
            for blk in make_tail(0, s2a, b2a):
                blk()

    for bi, sem, val in deferred_waits:
        patched = False
        for w in bi.ins.sync_info.on_wait:
            if w.id == sem.num and w.wait_value == 0:
                w.wait_value = val
                patched = True
                break
        assert patched, f"deferred wait not found on {bi.ins.name}"

    nc.compile()
    return nc


def kernel(x, W1, W2, gamma1, beta1, gamma2, beta2, mask1, mask2,
           _trace=False, _trace_kwargs=None):
    from concourse.bass_utils import run_bass_kernel_spmd

    K1, wt1a, wt1b, sel, wt2a, wt2b, aff = _pack(
        W1, W2, mask1, mask2, gamma1, beta1, gamma2, beta2)

    key = ("v2", K1)
    if key not in _cache:
        _cache[key] = _build(K1)
    nc = _cache[key]

    x = np.ascontiguousarray(np.asarray(x, np.float32))
    in_maps = [{"x": x[i * NL:(i + 1) * NL], "wt1a": wt1a, "wt1b": wt1b,
                "sel": sel, "wt2a": wt2a, "wt2b": wt2b, "aff": aff}
               for i in range(N_CORES)]
    kw = {}
    if _trace:
        kw = dict(trace=True, **(_trace_kwargs or {}))
    res = run_bass_kernel_spmd(nc, in_maps, core_ids=list(range(N_CORES)), **kw)
    out = np.concatenate([res.results[i]["out"] for i in range(N_CORES)], axis=0)
    _cache["last_results"] = res
    return out
